# revision 29
# baseline (speedup 1.0000x reference)
"""CausalAttNetV1 Trainium2 kernel.

Computes, per batch element b (one NeuronCore each, B=8 = n_cores):
    pr = x @ W1[:, :FD].T ; pc = x @ W1[:, FD:].T
    edge_scores[i,j] = W2 . relu(pr_i + pc_j + b1) + b2
    soft = softmax(edge_scores.flatten() / TEMP)
    thr  = k-th largest of soft (k = 0.6*N^2)
    causal = normalize(soft * (soft >= thr)) ; conf = 1 - causal

Strategy: data-parallel over B. Inside a core:
  - x transposed via PE (fp32 exact), projections via fp32 matmuls.
  - main loop over columns j: relu tiles [128h x 256i] in float32r produced
    by DVE (tensor_scalar add+max fused, 2x mode) and ACT (activation Relu +
    bias), reduced over h by PE flipped matmuls (relu tile stationary, W2
    hi/lo f32r pair moving -> scores land in natural [i, j] layout as
    even/odd PSUM columns, summed + b2 at eviction).
  - softmax with ACT exp+accum and gpsimd partition_all_reduce;
  - exact top-k threshold via counting bisection (26 iters; DVE
    is_ge+accum count, gpsimd all-reduce, replicated [128,1] state;
    bracket [0, min(1/Z, 1.00001/k)]); mask+renormalize; DMA out.

KSTAGE env (debug): 1=loads+transpose, 2=+projections, 3=+main loop,
4=+softmax, 5=full (default).
"""
import os
import sys
import numpy as np

sys.path.insert(0, "/opt/trn_rl_repo")
sys.path.insert(0, "/opt/trn_rl_repo/concourse")

import concourse.bacc as bacc
import concourse.mybir as mybir
import concourse.tile as tile
import concourse.bass_isa as bass_isa
from concourse.bass_utils import run_bass_kernel_spmd
from concourse.alu_op_type import AluOpType
from contextlib import ExitStack

F32 = mybir.dt.float32
F32R = mybir.dt.float32r
I32 = mybir.dt.int32
AF = mybir.ActivationFunctionType

B, N, FD, H = 8, 256, 1024, 256
TEMP = 0.5
K_TOP = int(0.6 * N * N)  # 39321
P = 128
NC2 = N // P              # 2 chunks of i/j/h
FC = FD // P              # 8 f-chunks
KSTAGE = int(os.environ.get("KSTAGE", "5"))
BISECT_ITERS = int(os.environ.get("KBISECT", "25"))
ACT_SHARE = int(os.environ.get("KACT_SHARE", "171"))
PROJ_F32R = os.environ.get("KPROJ", "f32r") == "f32r"


def _build():
    nc = bacc.Bacc("TRN2", target_bir_lowering=False, debug=False,
                   enable_asserts=True, num_devices=B)

    x_d = nc.dram_tensor("x", [N, FD], F32, kind="ExternalInput").ap()
    w1rt_d = nc.dram_tensor("w1rt", [FD, H], F32, kind="ExternalInput").ap()
    w1ct_d = nc.dram_tensor("w1ct", [FD, H], F32, kind="ExternalInput").ap()
    b1_d = nc.dram_tensor("b1", [H], F32, kind="ExternalInput").ap()
    w2_d = nc.dram_tensor("w2", [H], F32, kind="ExternalInput").ap()
    b2_d = nc.dram_tensor("b2", [1], F32, kind="ExternalInput").ap()

    esc_d = nc.dram_tensor("esc", [N, N], F32, kind="ExternalOutput").ap()
    soft_d = nc.dram_tensor("soft", [N * N], F32, kind="ExternalOutput").ap()
    causal_d = nc.dram_tensor("causal", [N, N], F32, kind="ExternalOutput").ap()
    conf_d = nc.dram_tensor("conf", [N, N], F32, kind="ExternalOutput").ap()

    _emit(nc, x_d, w1rt_d, w1ct_d, b1_d, w2_d, b2_d, esc_d, soft_d, causal_d, conf_d)
    nc.compile()
    return nc


def _emit(nc, x_d, w1rt_d, w1ct_d, b1_d, w2_d, b2_d, esc_d, soft_d, causal_d, conf_d):
    with tile.TileContext(nc) as tc, ExitStack() as ctx:
        big = ctx.enter_context(tc.tile_pool(name="big", bufs=1))
        rts = ctx.enter_context(tc.tile_pool(name="rts", bufs=int(os.environ.get("KRTBUFS", "6"))))
        small = ctx.enter_context(tc.tile_pool(name="small", bufs=1))
        ps_sc = ctx.enter_context(tc.tile_pool(name="ps_sc", bufs=1, space="PSUM"))
        ps_tp = ctx.enter_context(tc.tile_pool(name="ps_tp", bufs=2, space="PSUM"))
        ps_pj = ctx.enter_context(tc.tile_pool(name="ps_pj", bufs=2, space="PSUM"))
        ps_pr = ctx.enter_context(tc.tile_pool(name="ps_pr", bufs=1, space="PSUM"))

        # ---------------- loads ----------------
        x_sb = big.tile([P, NC2, FD], F32)          # [p, nchunk, f]
        for m in range(NC2):
            nc.sync.dma_start(out=x_sb[:, m, :], in_=x_d[m * P:(m + 1) * P, :])
        w1rt_sb = big.tile([P, FC, H], F32)         # [p(f), fchunk, h]
        w1ct_sb = big.tile([P, FC, H], F32)
        for c0, c1 in ((0, 4), (4, 8)):
            rows = slice(c0 * P, c1 * P)
            nc.sync.dma_start(
                out=w1rt_sb[:, c0:c1, :],
                in_=w1rt_d[rows, :].rearrange("(c p) h -> p c h", p=P))
            nc.sync.dma_start(
                out=w1ct_sb[:, c0:c1, :],
                in_=w1ct_d[rows, :].rearrange("(c p) h -> p c h", p=P))
        b1_sb = small.tile([P, NC2], F32)
        nc.sync.dma_start(out=b1_sb, in_=b1_d.rearrange("(c p) -> p c", p=P))
        w2_sb = small.tile([P, NC2], F32)
        nc.sync.dma_start(out=w2_sb, in_=w2_d.rearrange("(c p) -> p c", p=P))
        b2_sb = small.tile([P, 1], F32)
        nc.sync.dma_start(out=b2_sb, in_=b2_d.to_broadcast([P, 1]))

        # w2 split into f32r hi/lo pairs: [p(h), hc, 2] with col0=hi, col1=residual
        # (scores accumulate into even/odd psum columns; summed at eviction ->
        # exact-w2 contribution, halving the f32r matmul error)
        w2r = small.tile([P, NC2, 2], F32R)
        for hc in range(NC2):
            nc.vector.tensor_copy(w2r[:, hc, 0:1], w2_sb[:, hc:hc + 1])
            nc.vector.tensor_tensor(w2r[:, hc, 1:2], w2_sb[:, hc:hc + 1],
                                    w2r[:, hc, 0:1].bitcast(F32),
                                    op=AluOpType.subtract)

        # identity for PE transposes
        coli = small.tile([P, P], I32)
        rowi = small.tile([P, 1], I32)
        colf = small.tile([P, P], F32)
        rowf = small.tile([P, 1], F32)
        ident = small.tile([P, P], F32)
        nc.gpsimd.iota(coli, pattern=[[1, P]], base=0, channel_multiplier=0)
        nc.gpsimd.iota(rowi, pattern=[[0, 1]], base=0, channel_multiplier=1)
        nc.vector.tensor_copy(colf, coli)
        nc.vector.tensor_copy(rowf, rowi)
        nc.vector.tensor_scalar(ident, colf, rowf[:, 0:1], None, op0=AluOpType.is_equal)

        # ---------------- transpose x ----------------
        xt_dt = F32R if PROJ_F32R else F32
        xt = big.tile([P, FC, N], xt_dt)            # [p(f), fchunk, n]
        for c in range(FC):
            for m in range(NC2):
                pt = ps_tp.tile([P, P], F32, tag="tp")
                nc.tensor.transpose(pt, x_sb[:, m, c * P:(c + 1) * P], ident)
                if (c * NC2 + m) % 2 == 0:
                    nc.scalar.copy(xt[:, c, m * P:(m + 1) * P], pt)
                else:
                    nc.vector.tensor_copy(xt[:, c, m * P:(m + 1) * P], pt)

        if KSTAGE < 2:
            for ic in range(NC2):
                nc.sync.dma_start(out=esc_d[ic * P:(ic + 1) * P, :],
                                  in_=xt[:, ic, :].bitcast(F32))
            return

        # ---------------- projections ----------------
        # A[h, i] = pr^T + b1 ; pct[h, j] = pc^T
        if PROJ_F32R:
            w1rt_r = big.tile([P, FC, H], F32R)
            w1ct_r = big.tile([P, FC, H], F32R)
            for c in range(FC):
                nc.vector.tensor_copy(w1rt_r[:, c, :], w1rt_sb[:, c, :])
                nc.vector.tensor_copy(w1ct_r[:, c, :], w1ct_sb[:, c, :])
            w1rt_mm, w1ct_mm = w1rt_r, w1ct_r
        else:
            w1rt_mm, w1ct_mm = w1rt_sb, w1ct_sb
        xt_mm = xt
        # b1 folded into pct (so both producer variants use the same scalar);
        # pr kept raw in A_sb (SBUF, for DVE) AND in PSUM (for ACT's faster port)
        A_sb = big.tile([P, NC2, N], F32)
        pct_sb = big.tile([P, NC2, N], F32)
        pr_ps0 = ps_pr.tile([P, N], F32, tag="pr0")
        pr_ps1 = ps_pr.tile([P, N], F32, tag="pr1")
        pr_ps = [pr_ps0, pr_ps1]
        for hc in range(NC2):
            for c in range(FC):
                nc.tensor.matmul(pr_ps[hc], w1rt_mm[:, c, hc * P:(hc + 1) * P],
                                 xt_mm[:, c, :],
                                 start=(c == 0), stop=(c == FC - 1))
            nc.scalar.copy(A_sb[:, hc, :], pr_ps[hc])
            pj = ps_pj.tile([P, N], F32, tag="pj")
            for c in range(FC):
                nc.tensor.matmul(pj, w1ct_mm[:, c, hc * P:(hc + 1) * P], xt_mm[:, c, :],
                                 start=(c == 0), stop=(c == FC - 1))
            nc.scalar.activation(pct_sb[:, hc, :], pj, AF.Identity,
                                 bias=b1_sb[:, hc:hc + 1], scale=1.0)

        if KSTAGE < 3:
            for ic in range(NC2):
                nc.sync.dma_start(out=esc_d[ic * P:(ic + 1) * P, :], in_=A_sb[:, ic, :])
                nc.sync.dma_start(out=causal_d[ic * P:(ic + 1) * P, :], in_=pct_sb[:, ic, :])
            return

        # ---------------- main loop ----------------
        # relu tiles over [h, i] for fixed column j; scores land natural [i, j]
        sc_ps0 = ps_sc.tile([P, 2 * N], F32, tag="sc0")
        sc_ps1 = ps_sc.tile([P, 2 * N], F32, tag="sc1")
        sc_ps = [sc_ps0, sc_ps1]
        for j in range(N):
            rt = []
            for hc in range(NC2):
                t = rts.tile([P, N], F32R, tag=f"rt{hc}")
                c = j * NC2 + hc
                if (c * ACT_SHARE) // 512 != ((c - 1) * ACT_SHARE) // 512:
                    nc.scalar.activation(t, pr_ps[hc], AF.Relu,
                                         bias=pct_sb[:, hc, j:j + 1], scale=1.0)
                else:
                    nc.vector.tensor_scalar(t, A_sb[:, hc, :],
                                            pct_sb[:, hc, j:j + 1], 0.0,
                                            op0=AluOpType.add, op1=AluOpType.max)
                rt.append(t)
            for ic in range(NC2):
                for hc in range(NC2):
                    nc.tensor.matmul(sc_ps[ic][:, 2 * j:2 * j + 2],
                                     rt[hc][:, ic * P:(ic + 1) * P],
                                     w2r[:, hc, :],
                                     start=(hc == 0), stop=(hc == 1))

        # evict scores (+b2): hi column + lo column + b2, natural [i, j] layout
        scores = big.tile([P, NC2, N], F32)         # [p(i), ic, j]
        for ic in range(NC2):
            pshl = sc_ps[ic].rearrange("p (j two) -> p two j", two=2)
            nc.scalar.activation(
                scores[:, ic, :], pshl[:, 0, :],
                AF.Identity, bias=b2_sb[:, 0:1], scale=1.0)
            nc.vector.tensor_tensor(scores[:, ic, :], scores[:, ic, :],
                                    pshl[:, 1, :], op=AluOpType.add)
        for ic in range(NC2):
            nc.sync.dma_start(out=esc_d[ic * P:(ic + 1) * P, :], in_=scores[:, ic, :])

        if KSTAGE < 4:
            return

        # ---------------- softmax ----------------
        scf = scores.rearrange("p c n -> p (c n)")
        rowmax = small.tile([P, 1], F32)
        nc.vector.tensor_reduce(rowmax, scf, axis=mybir.AxisListType.X,
                                op=AluOpType.max)
        gmax = small.tile([P, 1], F32)
        nc.gpsimd.partition_all_reduce(gmax, rowmax, channels=P,
                                       reduce_op=bass_isa.ReduceOp.max)
        negm2 = small.tile([P, 1], F32)
        nc.vector.tensor_scalar_mul(negm2, gmax, -1.0 / TEMP)
        expv = big.tile([P, NC2, N], F32)
        esum = small.tile([P, 1], F32)
        nc.scalar.activation(expv.rearrange("p c n -> p (c n)"), scf, AF.Exp,
                             bias=negm2[:, 0:1], scale=1.0 / TEMP, accum_out=esum)
        zsum = small.tile([P, 1], F32)
        nc.gpsimd.partition_all_reduce(zsum, esum, channels=P,
                                       reduce_op=bass_isa.ReduceOp.add)
        rz = small.tile([P, 1], F32)
        nc.vector.reciprocal(rz, zsum)
        soft = big.tile([P, NC2, N], F32)
        nc.vector.tensor_scalar(soft.rearrange("p c n -> p (c n)"),
                                expv.rearrange("p c n -> p (c n)"),
                                rz[:, 0:1], None, op0=AluOpType.mult)
        soft3 = soft_d.rearrange("(c p n) -> c p n", c=NC2, p=P)
        for ic in range(NC2):
            nc.sync.dma_start(out=soft3[ic], in_=soft[:, ic, :])

        if KSTAGE < 5:
            return

        # ---------------- top-k threshold: counting bisection ----------------
        # State replicated across partitions [128,1]; global count via gpsimd
        # partition_all_reduce. hi starts at 1/Z = exact max of soft.
        softf = soft.rearrange("p c n -> p (c n)")
        lo = small.tile([P, 1], F32)
        hi = small.tile([P, 1], F32)
        mid = small.tile([P, 1], F32)
        sel = small.tile([P, 1], F32)
        tmp = small.tile([P, 1], F32)
        call = small.tile([P, 1], F32)
        nc.vector.memset(lo, 0.0)
        # Markov: k-th largest of values summing to ~1 is <= 1/k
        nc.vector.tensor_scalar_min(hi, rz, 1.00001 / K_TOP)
        nc.vector.tensor_scalar(mid, hi, 0.5, None, op0=AluOpType.mult)
        junk = big.tile([P, N * NC2], F32)
        csb = small.tile([P, 1], F32)
        for it in range(BISECT_ITERS):
            nc.vector.tensor_scalar(junk, softf, mid[:, 0:1], 0.0,
                                    op0=AluOpType.is_ge, op1=AluOpType.add,
                                    accum_out=csb)
            nc.gpsimd.partition_all_reduce(call, csb, channels=P,
                                           reduce_op=bass_isa.ReduceOp.add)
            # sel = 1 if count >= K ; lo = max(mid*sel, lo) ; hi = min(sel*BIG+mid, hi)
            nc.vector.tensor_scalar(sel, call, float(K_TOP), None,
                                    op0=AluOpType.is_ge)
            nc.vector.scalar_tensor_tensor(lo, mid, sel[:, 0:1], lo,
                                           op0=AluOpType.mult, op1=AluOpType.max)
            nc.vector.scalar_tensor_tensor(tmp, sel, 3.0e38, mid,
                                           op0=AluOpType.mult, op1=AluOpType.add)
            nc.vector.tensor_tensor(hi, hi, tmp, op=AluOpType.min)
            nc.vector.tensor_scalar(mid, lo, hi[:, 0:1], 0.5,
                                    op0=AluOpType.add, op1=AluOpType.mult)

        # ---------------- mask + normalize + outputs ----------------
        # per-ic chunks so chunk-0 normalize/DMA overlaps chunk-1 compute
        causal_un = big.tile([P, NC2, N], F32)
        csum = small.tile([P, NC2], F32)
        for ic in range(NC2):
            nc.vector.scalar_tensor_tensor(causal_un[:, ic, :],
                                           soft[:, ic, :], lo[:, 0:1], soft[:, ic, :],
                                           op0=AluOpType.is_ge, op1=AluOpType.mult,
                                           accum_out=csum[:, ic:ic + 1])
        csumt = small.tile([P, 1], F32)
        nc.vector.tensor_tensor(csumt, csum[:, 0:1], csum[:, 1:2], op=AluOpType.add)
        ssum = small.tile([P, 1], F32)
        nc.gpsimd.partition_all_reduce(ssum, csumt, channels=P,
                                       reduce_op=bass_isa.ReduceOp.add)
        ssel = small.tile([P, 1], F32)
        nc.vector.tensor_scalar(ssel, ssum, 1e-12, None, op0=AluOpType.add)
        rsel = small.tile([P, 1], F32)
        nc.vector.reciprocal(rsel, ssel)
        causal = big.tile([P, NC2, N], F32)
        conf = big.tile([P, NC2, N], F32)
        for ic in range(NC2):
            nc.vector.tensor_scalar(causal[:, ic, :], causal_un[:, ic, :],
                                    rsel[:, 0:1], None, op0=AluOpType.mult)
            nc.sync.dma_start(out=causal_d[ic * P:(ic + 1) * P, :], in_=causal[:, ic, :])
            nc.scalar.activation(conf[:, ic, :], causal[:, ic, :], AF.Identity,
                                 bias=1.0, scale=-1.0)
            nc.sync.dma_start(out=conf_d[ic * P:(ic + 1) * P, :], in_=conf[:, ic, :])


_NC_CACHE = None


def _get_nc():
    global _NC_CACHE
    if _NC_CACHE is None:
        _NC_CACHE = _build()
    return _NC_CACHE


def kernel(x, W1, b1, W2, b2, _trace=False, _trace_kwargs=None):
    x = np.ascontiguousarray(np.asarray(x, dtype=np.float32))
    W1 = np.asarray(W1, dtype=np.float32)
    b1 = np.ascontiguousarray(np.asarray(b1, dtype=np.float32))
    W2 = np.asarray(W2, dtype=np.float32)
    b2 = np.ascontiguousarray(np.asarray(b2, dtype=np.float32))
    w1rt = np.ascontiguousarray(W1[:, :FD].T)
    w1ct = np.ascontiguousarray(W1[:, FD:].T)
    w2v = np.ascontiguousarray(W2[0])

    nc = _get_nc()
    in_maps = [{"x": np.ascontiguousarray(x[b]), "w1rt": w1rt, "w1ct": w1ct,
                "b1": b1, "w2": w2v, "b2": b2} for b in range(B)]
    res = run_bass_kernel_spmd(nc, in_maps, core_ids=list(range(B)),
                               trace=_trace, **(_trace_kwargs or {}))
    causal = np.stack([res.results[b]["causal"] for b in range(B)])
    conf = np.stack([res.results[b]["conf"] for b in range(B)])
    esc = np.stack([res.results[b]["esc"] for b in range(B)])
    soft = np.stack([res.results[b]["soft"] for b in range(B)])
    kernel._last_results = res
    return causal, conf, esc, soft


kernel._last_results = None

